# revision 39
# baseline (speedup 1.0000x reference)
"""CVTGAD loss kernel for 8 TRN2 NeuronCores (v3 -- host-transposed loads).

Math (matches the jax reference):
  l_node[b] = mean_i [ lse_j(sim_ij) - sim_ii ]   per graph (128x128 InfoNCE)
  l_graph   = InfoNCE over pooled graph embeddings (512x512)
  out = (std(l_node)+1e-6) * mean(l_node) + (std(l_graph)+1e-6) * mean(l_graph)

Sharding: 64 graphs (8192 node rows) per core; h_s_final replicated (rolled per
core so each core's own graphs sit at columns 0:64). Host stages each shard
PRE-TRANSPOSED (d-major, bf16) so the device does plain HWDGE loads -- no
SWDGE casts, no bulk on-device transposes (v2 spent ~29us of DMA there).

Per-core dataflow (all tiles d-on-partitions [128, chunk, nodes]):
  - sq_f/sq_s/prod: DVE tensor_tensor squares + cross products (2x mode).
  - Per-node stats via PE ones-matmuls (partition reduction is ~free on PE):
    ssq_f/ssq_s/pos_raw land as [128 nodes, graphs] f32 columns in PSUM.
  - ACT Ln+Exp -> c_i = (1/tau)/|hf_i| (bf16) and v_j = 1/|hs_j| (f32 cols).
  - c to free-layout: PE-transpose (identity matmul) -> DMA concat -> one
    stride-0-free DMA broadcast to all 128 partitions -> DVE folds c into hf.
  - Per-graph gram S^T[j,i] = hsT^T @ hfc -> PSUM; ACT Exp(scale=v_j);
    rowsum_i via ones-matmul; epilogue ln(rowsum) - pos*c*v columns.
  - Graph-level 512-wide InfoNCE with the same pattern (c_f per-partition via
    DVE bcast-col mult, per-128-chunk PE-transposes, 4 exps, accumulated
    ones-matmul rowsum).
Host epilogue: means/std + weighted sum (tiny, f64).
"""

import numpy as np

B = 512
NPER = 128
D = 256
NCORES = 8
GPC = B // NCORES      # 64 graphs per core
import os as _os
_blks = _os.environ.get("K_BLKS", "16,16,16,16")
BSZ = [int(x) for x in _blks.split(",")]
assert sum(BSZ) == GPC
NBLK = len(BSZ)
BOFF = [sum(BSZ[:i]) for i in range(NBLK)]
BMAX = max(BSZ)
FMAX = BMAX * NPER
TAU = 0.5
LN_INV_TAU = float(np.log(1.0 / TAU))

_CACHE = {}


def _build():
    import os
    import concourse.bacc as bacc
    import concourse.tile as tile
    import concourse.mybir as mybir
    import concourse.hw_specs as hw_specs
    from concourse.bass import AP
    from concourse._compat import get_trn_type

    # Pin every activation to the one table set that has Exp+Ln+Square+Copy,
    # so the compiler emits a single ACT_TABLE_LOAD instead of thrashing.
    if not getattr(hw_specs, "_nle_patched", False):
        _orig_tables = hw_specs.get_activation_tables

        def _only_nle(arch):
            t = _orig_tables(arch)
            keep = "natural_log_exp_and_others"
            return {k: (v if k == keep else set()) for k, v in t.items()}

        hw_specs.get_activation_tables = _only_nle
        bacc.get_activation_tables = _only_nle
        hw_specs._nle_patched = True

    f32 = mybir.dt.float32
    bf16 = mybir.dt.bfloat16
    AF = mybir.ActivationFunctionType
    ALU = mybir.AluOpType

    # Engine-balance knobs: how many blocks' square passes run on ACT
    # (Square activation) instead of DVE tensor_tensor.
    # chunks (0-2) of each square pass that run on ACT instead of DVE
    SQF_ACT_C = int(os.environ.get("K_SQF_ACT_C", "0"))
    SQS_ACT_C = int(os.environ.get("K_SQS_ACT_C", "1"))
    # of the 4 hfc quarter-passes, how many run on Pool
    HFC_POOL_N = int(os.environ.get("K_HFC_POOL_N", "0"))
    LAG = int(os.environ.get("K_LAG", "2"))

    nc = bacc.Bacc(get_trn_type() or "TRN2", target_bir_lowering=False, debug=True)

    hft = nc.declare_dram_parameter("hft", [D, GPC * NPER], bf16, isOutput=False)
    hst = nc.declare_dram_parameter("hst", [D, GPC * NPER], bf16, isOutput=False)
    hfft = nc.declare_dram_parameter("hfft", [D, GPC], bf16, isOutput=False)
    hsft = nc.declare_dram_parameter("hsft", [D, B], bf16, isOutput=False)
    ident = nc.declare_dram_parameter("ident", [128, 128], bf16, isOutput=False)
    out_node = nc.declare_dram_parameter("out_node", [NPER, GPC], f32, isOutput=True)
    out_graph = nc.declare_dram_parameter("out_graph", [GPC, 1], f32, isOutput=True)

    hft_r = hft[:, :].rearrange("(c p) f -> p c f", p=128)
    hst_r = hst[:, :].rearrange("(c p) f -> p c f", p=128)

    with tile.TileContext(nc) as tc:
        with (
            tc.tile_pool(name="consts", bufs=1) as consts,
            tc.tile_pool(name="cols", bufs=1) as colsp,
            tc.tile_pool(name="loads", bufs=int(os.environ.get("K_LOADS", "3"))) as loads,
            tc.tile_pool(name="scr", bufs=int(os.environ.get("K_SCR", "2"))) as scr,
            tc.tile_pool(name="hfcp", bufs=int(os.environ.get("K_HFC", "2"))) as hfcp,
            tc.tile_pool(name="cbp", bufs=2) as cbp,
            tc.tile_pool(name="ep", bufs=int(os.environ.get("K_EP", "4"))) as ep,
            tc.tile_pool(name="bigps", bufs=1, space="PSUM") as bigps,
            tc.tile_pool(name="ctps", bufs=1, space="PSUM") as ctps,
            tc.tile_pool(name="spsum", bufs=int(os.environ.get("K_SPSUM", "3")), space="PSUM") as spsum,
        ):
            ones_c = consts.tile([128, 1], bf16)
            nc.vector.memset(ones_c, 1.0)
            lnk_c = consts.tile([128, 1], f32)
            nc.vector.memset(lnk_c, LN_INV_TAU)
            ident_sb = consts.tile([128, 128], bf16)
            nc.scalar.dma_start(out=ident_sb, in_=ident[:, :])

            # persistent column stats [128 nodes, GPC graphs]
            c_all = colsp.tile([128, GPC], bf16)
            v_all = colsp.tile([128, GPC], f32)
            lnf_scr = colsp.tile([128, GPC], f32)
            lns_scr = colsp.tile([128, GPC], f32)
            lsum_c = colsp.tile([128, GPC], f32)
            w_c = colsp.tile([128, GPC], f32)
            posv_c = colsp.tile([128, GPC], f32)
            l_cols = colsp.tile([128, GPC], f32)

            # one PSUM bank packs rowsum (0:64), pos (64:128), and the
            # double-buffered per-block ssf/sss stats (128:192).
            acc_ps = bigps.tile([128, 192], f32)
            rowsum_ps = acc_ps[:, 0:GPC]
            pos_ps = acc_ps[:, GPC : 2 * GPC]
            # one bank for the double-buffered transposed-c tiles
            ct_ps = ctps.tile([BMAX, 2, 128], bf16)

            hfT_t, hsT_t, hfc_t = {}, {}, {}

            def stage_load(b):
                FREE = BSZ[b] * NPER
                fs = slice(BOFF[b] * NPER, BOFF[b] * NPER + FREE)
                hfT_full = loads.tile([128, 2, FMAX], bf16, tag="hfT")
                hfT = hfT_full[:, :, :FREE]
                hsT_full = loads.tile([128, 2, FMAX], bf16, tag="hsT")
                hsT = hsT_full[:, :, :FREE]
                if b == 0:
                    # per-chunk loads so block 0's first square starts as
                    # soon as the first quarter of the data lands.
                    for c in range(2):
                        nc.sync.dma_start(out=hfT[:, c, :], in_=hft_r[:, c, fs])
                    for c in range(2):
                        nc.sync.dma_start(out=hsT[:, c, :], in_=hst_r[:, c, fs])
                else:
                    nc.sync.dma_start(out=hfT, in_=hft_r[:, :, fs])
                    nc.sync.dma_start(out=hsT, in_=hst_r[:, :, fs])
                hfT_t[b] = hfT
                hsT_t[b] = hsT

            def stage_stats(b):
                BLK = BSZ[b]
                bs = slice(BOFF[b], BOFF[b] + BLK)
                hfT, hsT = hfT_t[b], hsT_t[b]

                soff = 2 * GPC + (b % 2) * 2 * BMAX
                ssf_ps = acc_ps[:, soff : soff + BLK]
                sss_ps = acc_ps[:, soff + BLK : soff + 2 * BLK]

                # DVE order is crafted gap-free: sq_f, sq_s, c_rt, prod, hfc.
                # The c-chain's PE-T/ACT/DMA hops hide under sq_s and prod.
                sq_f_full = scr.tile([128, 2, FMAX], bf16, tag="sq_f")
                sq_f = sq_f_full[:, :, : BLK * NPER]
                for c in range(2):
                    if c < SQF_ACT_C and b < NBLK - 2:
                        nc.scalar.activation(sq_f[:, c, :], hfT[:, c, :], AF.Square)
                    else:
                        nc.vector.tensor_tensor(
                            sq_f[:, c, :], hfT[:, c, :], hfT[:, c, :], op=ALU.mult
                        )
                with tc.high_priority():
                    for g in range(BLK):
                        gs = slice(g * NPER, (g + 1) * NPER)
                        for c in range(2):
                            nc.tensor.matmul(
                                ssf_ps[:, g : g + 1], sq_f[:, c, gs], ones_c,
                                start=(c == 0), stop=(c == 1),
                            )
                    # c = exp(-0.5*ln(ssf) + ln(1/tau)) = (1/tau)/|hf| (bf16)
                    nc.scalar.activation(lnf_scr[:, bs], ssf_ps, AF.Ln)
                    nc.scalar.activation(
                        c_all[:, bs], lnf_scr[:, bs], AF.Exp, scale=-0.5, bias=lnk_c
                    )
                sq_s_full = scr.tile([128, 2, FMAX], bf16, tag="sq_s")
                sq_s = sq_s_full[:, :, : BLK * NPER]
                for c in range(2):
                    if c < SQS_ACT_C and b < NBLK - 2 and BSZ[b] == BMAX:
                        nc.scalar.activation(sq_s[:, c, :], hsT[:, c, :], AF.Square)
                    else:
                        nc.vector.tensor_tensor(
                            sq_s[:, c, :], hsT[:, c, :], hsT[:, c, :], op=ALU.mult
                        )
                for g in range(BLK):
                    gs = slice(g * NPER, (g + 1) * NPER)
                    for c in range(2):
                        nc.tensor.matmul(
                            sss_ps[:, g : g + 1], sq_s[:, c, gs], ones_c,
                            start=(c == 0), stop=(c == 1),
                        )
                # v = 1/|hs|  (f32, exp-scale slices)
                nc.scalar.activation(lns_scr[:, bs], sss_ps, AF.Ln)
                nc.scalar.activation(
                    v_all[:, bs], lns_scr[:, bs], AF.Exp, scale=-0.5
                )

                prod_full = scr.tile([128, 2, FMAX], bf16, tag="prod")
                prod = prod_full[:, :, : BLK * NPER]
                nc.vector.tensor_tensor(prod, hfT, hsT, op=ALU.mult)
                for g in range(BLK):
                    gg = BOFF[b] + g
                    gs = slice(g * NPER, (g + 1) * NPER)
                    for c in range(2):
                        nc.tensor.matmul(
                            pos_ps[:, gg : gg + 1], prod[:, c, gs], ones_c,
                            start=(c == 0), stop=(c == 1),
                        )

                # transposed-c lands in PSUM at the tail of this block's PE
                # work; the cchain stage picks it up next iteration.
                c_colsT = ct_ps[:BLK, b % 2, :]
                with tc.high_priority():
                    nc.tensor.transpose(c_colsT, c_all[:, bs], ident_sb)

            def stage_cchain(b):
                BLK = BSZ[b]
                FREE = BLK * NPER
                hfT = hfT_t.pop(b)
                hp = tc.high_priority()
                hp.__enter__()
                c_colsT = ct_ps[:BLK, b % 2, :]
                # PSUM -> SBUF hop (HWDGE cannot read PSUM), then concat and
                # a stride-0-repeat bcast across partitions.
                c_rt_full = cbp.tile([BMAX, 128], bf16, tag="c_rt")
                c_rt = c_rt_full[:BLK, :]
                nc.vector.tensor_copy(c_rt, c_colsT)
                c_row_full = cbp.tile([1, BMAX, 128], bf16, tag="c_row")
                c_row = c_row_full[:, :BLK, :]
                nc.scalar.dma_start(out=c_row, in_=c_rt)
                c_full_t = cbp.tile([128, FMAX], bf16, tag="c_full")
                c_full = c_full_t[:, :FREE]
                crow_ap = c_row[:, :, :]
                bcast_src = AP(crow_ap.tensor, crow_ap.offset, [[1, 1], [0, 128], [1, FREE]])
                nc.scalar.dma_start(out=c_full, in_=bcast_src)

                hfc_full = hfcp.tile([128, 2, FMAX], bf16, tag="hfc")
                hfc = hfc_full[:, :, :FREE]
                HF2 = FREE // 2
                k = 0
                for h in range(2):
                    hs_ = slice(h * HF2, (h + 1) * HF2)
                    for c in range(2):
                        eng = nc.gpsimd if k < HFC_POOL_N else nc.vector
                        eng.tensor_tensor(
                            hfc[:, c, hs_], hfT[:, c, hs_], c_full[:, hs_],
                            op=ALU.mult,
                        )
                        k += 1
                hp.__exit__(None, None, None)
                hfc_t[b] = hfc

            def stage_gram(b):
                BLK = BSZ[b]
                hsT = hsT_t.pop(b)
                hfc = hfc_t.pop(b)
                # per-graph gram->exp->rowsum; rowsum lagged so PE never
                # queue-waits on an exp that hasn't run yet.
                RLAG = 2
                eT_t = {}
                for g in range(BLK + RLAG):
                    if g < BLK:
                        gs = slice(g * NPER, (g + 1) * NPER)
                        gg = BOFF[b] + g
                        s_ps = spsum.tile([128, 128], f32, tag="s_ps")
                        for c in range(2):
                            nc.tensor.matmul(
                                s_ps, hsT[:, c, gs], hfc[:, c, gs],
                                start=(c == 0), stop=(c == 1),
                            )
                        eT = ep.tile([128, 128], bf16, tag="eT")
                        nc.scalar.activation(
                            eT, s_ps, AF.Exp, scale=v_all[:, gg : gg + 1]
                        )
                        eT_t[g] = eT
                    if g >= RLAG:
                        gl = g - RLAG
                        nc.tensor.matmul(
                            rowsum_ps[:, BOFF[b] + gl : BOFF[b] + gl + 1],
                            eT_t.pop(gl), ones_c,
                            start=True, stop=True,
                        )
                bs_ = slice(BOFF[b], BOFF[b] + BLK)
                nc.scalar.activation(lsum_c[:, bs_], rowsum_ps[:, bs_], AF.Ln)
                nc.vector.tensor_tensor(w_c[:, bs_], c_all[:, bs_], v_all[:, bs_], op=ALU.mult)
                nc.vector.tensor_tensor(posv_c[:, bs_], pos_ps[:, bs_], w_c[:, bs_], op=ALU.mult)
                nc.vector.tensor_tensor(l_cols[:, bs_], lsum_c[:, bs_], posv_c[:, bs_], op=ALU.subtract)
                nc.sync.dma_start(out=out_node[:, bs_], in_=l_cols[:, bs_])

            gl_t = {}

            def graph_loads(fin):
                hffT = fin.tile([128, 2, GPC], bf16)
                nc.scalar.dma_start(
                    out=hffT, in_=hfft[:, :].rearrange("(c p) f -> p c f", p=128)
                )
                hsfT = fin.tile([128, 2, B], bf16)
                nc.scalar.dma_start(
                    out=hsfT, in_=hsft[:, :].rearrange("(c p) f -> p c f", p=128)
                )
                gl_t["hffT"] = hffT
                gl_t["hsfT"] = hsfT

            def graph_level(fin, fpsum):
                # pooled-graph InfoNCE: own 64 rows x all 512 cols
                if True:
                    hffT = gl_t.pop("hffT")
                    hsfT = gl_t.pop("hsfT")

                    sq_ff = fin.tile([128, 2, GPC], bf16)
                    nc.vector.tensor_tensor(sq_ff, hffT, hffT, op=ALU.mult)
                    sq_sf = fin.tile([128, 2, B], bf16)
                    nc.vector.tensor_tensor(sq_sf, hsfT, hsfT, op=ALU.mult)
                    prod_f = fin.tile([128, 2, GPC], bf16)
                    nc.vector.tensor_tensor(
                        prod_f, hffT, hsfT[:, :, :GPC], op=ALU.mult
                    )

                    fsmall = fpsum.tile([128, 8], f32)
                    ssff_ps = fsmall[:GPC, 0:1]
                    vf_ps = fsmall[:, 1:5]
                    posf_ps = fsmall[:GPC, 5:6]
                    rsf_ps = fsmall[:GPC, 6:7]
                    for c in range(2):
                        nc.tensor.matmul(ssff_ps, sq_ff[:, c, :], ones_c,
                                         start=(c == 0), stop=(c == 1))
                    for q in range(4):
                        qs = slice(q * 128, (q + 1) * 128)
                        for c in range(2):
                            nc.tensor.matmul(vf_ps[:, q : q + 1], sq_sf[:, c, qs],
                                             ones_c, start=(c == 0), stop=(c == 1))
                    for c in range(2):
                        nc.tensor.matmul(posf_ps, prod_f[:, c, :], ones_c,
                                         start=(c == 0), stop=(c == 1))

                    lnff = fin.tile([GPC, 1], f32)
                    nc.scalar.activation(lnff, ssff_ps, AF.Ln)
                    c_f = fin.tile([GPC, 1], f32)
                    nc.scalar.activation(c_f, lnff, AF.Exp, scale=-0.5,
                                         bias=lnk_c[:GPC])
                    lnvf = fin.tile([128, 4], f32)
                    nc.scalar.activation(lnvf, vf_ps, AF.Ln)
                    v_f = fin.tile([128, 4], f32)
                    nc.scalar.activation(v_f, lnvf, AF.Exp, scale=-0.5)

                    gram_f = fpsum.tile([GPC, B], f32)
                    for c in range(2):
                        nc.tensor.matmul(gram_f, hffT[:, c, :], hsfT[:, c, :],
                                         start=(c == 0), stop=(c == 1))
                    # fold c_f (per-partition of gram_f) via bcast-col mult
                    svf = fin.tile([GPC, B], bf16)
                    cfap = c_f[:, :]
                    cfb = AP(cfap.tensor, cfap.offset, [[1, GPC], [0, B]])
                    nc.vector.tensor_tensor(svf, gram_f, cfb, op=ALU.mult)

                    svfT = fpsum.tile([128, 4, GPC], bf16)
                    for q in range(4):
                        qs = slice(q * 128, (q + 1) * 128)
                        nc.tensor.transpose(svfT[:, q, :], svf[:, qs], ident_sb[:GPC, :GPC])
                    e_f = fin.tile([128, 4, GPC], bf16)
                    for q in range(4):
                        nc.scalar.activation(e_f[:, q, :], svfT[:, q, :], AF.Exp,
                                             scale=v_f[:, q : q + 1])
                    for q in range(4):
                        nc.tensor.matmul(rsf_ps, e_f[:, q, :], ones_c,
                                         start=(q == 0), stop=(q == 3))

                    lnr_f = fin.tile([GPC, 1], f32)
                    nc.scalar.activation(lnr_f, rsf_ps, AF.Ln)
                    wf = fin.tile([GPC, 1], f32)
                    nc.vector.tensor_tensor(wf, c_f, v_f[:GPC, 0:1], op=ALU.mult)
                    posx = fin.tile([GPC, 1], f32)
                    nc.vector.tensor_tensor(posx, posf_ps, wf, op=ALU.mult)
                    lg = fin.tile([GPC, 1], f32)
                    nc.vector.tensor_tensor(lg, lnr_f, posx, op=ALU.subtract)
                    nc.sync.dma_start(out=out_graph[:, :], in_=lg)

            # ---- software-pipelined main loop ----
            GLI = int(os.environ.get("K_GLI", "2"))
            with (
                tc.tile_pool(name="fin", bufs=1) as fin,
                tc.tile_pool(name="fpsum", bufs=1, space="PSUM") as fpsum,
            ):
                # per-iteration issue order: the cchain of the block about to
                # gram goes FIRST on every queue (its deps are an iteration
                # old), then loads, then grams of the block-before, then
                # stats of the freshly loading block.
                for i in range(NBLK + 2):
                    if 1 <= i <= NBLK:
                        stage_cchain(i - 1)
                    if i < NBLK:
                        stage_load(i)
                    if i == 1:
                        graph_loads(fin)
                    if i >= 2:
                        stage_gram(i - 2)
                    if i == GLI:
                        graph_level(fin, fpsum)
                    if i < NBLK:
                        stage_stats(i)


    nc.compile()
    return nc


def _get_nc():
    if "nc" not in _CACHE:
        _CACHE["nc"] = _build()
    return _CACHE["nc"]


def _run(in_maps, **kwargs):
    from concourse.bass_utils import run_bass_kernel_spmd

    return run_bass_kernel_spmd(_get_nc(), in_maps, core_ids=list(range(NCORES)), **kwargs)


def make_in_maps(h_f_final, h_s_final, h_f, h_s):
    import ml_dtypes

    bf = ml_dtypes.bfloat16
    h_f = np.asarray(h_f, dtype=np.float32)
    h_s = np.asarray(h_s, dtype=np.float32)
    h_f_final = np.asarray(h_f_final, dtype=np.float32)
    h_s_final = np.asarray(h_s_final, dtype=np.float32)
    rows = GPC * NPER
    idv = np.eye(128, dtype=bf)
    in_maps = []
    for c in range(NCORES):
        hfs = h_f[c * rows : (c + 1) * rows]
        hss = h_s[c * rows : (c + 1) * rows]
        hffs = h_f_final[c * GPC : (c + 1) * GPC]
        hsfs = np.roll(h_s_final, -GPC * c, axis=0)
        in_maps.append(
            {
                "hft": np.ascontiguousarray(hfs.T).astype(bf),
                "hst": np.ascontiguousarray(hss.T).astype(bf),
                "hfft": np.ascontiguousarray(hffs.T).astype(bf),
                "hsft": np.ascontiguousarray(hsfs.T).astype(bf),
                "ident": idv,
            }
        )
    return in_maps


def finish(results):
    l_node = np.concatenate(
        [r["out_node"].astype(np.float64).mean(axis=0) for r in results]
    )
    l_graph = np.concatenate([r["out_graph"][:, 0].astype(np.float64) for r in results])
    lam1 = l_node.std() + 1e-6
    lam2 = l_graph.std() + 1e-6
    return np.float32(lam1 * l_node.mean() + lam2 * l_graph.mean())


def kernel(h_f_final, h_s_final, h_f, h_s, batch=None, **_unused):
    res = _run(make_in_maps(h_f_final, h_s_final, h_f, h_s))
    return finish(res.results)
